# revision 1
# baseline (speedup 1.0000x reference)
"""Cross multihead attention (global/local masked head groups) on 8 trn2 cores.

Sharding: core c -> (batch b = c//2, head-group g = c%2).
  g=0: heads 0-7  masked by key_padding_mask[b]
  g=1: heads 8-15 masked by local_mask[b]
Each core computes its group's partial output  (attn_out_g @ Wo[:, g*512:(g+1)*512].T)
of shape [T, E]; the host sums the two partials per batch and adds bo.

On-chip layout ("transposed scores" orientation - zero on-chip transposes):
  qT, kT   : [512(j), 1024(t|s)]  feature-major (j = head*64 + d)
  v        : [1024(s), 8*65]      natural, per-head 65-col stripes [v_h | ones]
  scoresT  : [s, t] tiles; mask folded into Exp bias (per-partition = per-s)
  softmax  : no max-subtraction (scores ~ N(0,1)); denominators from the
             ones column of the augmented v matmul; normalization deferred
             to a per-head [64, 512] multiply with a partition-broadcast
             reciprocal row.
  attnT    : [512(j), 1024(t)] -> out = attnT.T @ woT accumulated over j-tiles.
"""

import os
import sys

sys.path.insert(0, "/opt/trn_rl_repo")

import numpy as np

import concourse.bass as bass
import concourse.mybir as mybir
from concourse.tile import TileContext

B, T, S, E, H = 4, 1024, 1024, 1024, 16
DH = E // H            # 64
HH = H // 2            # 8 heads per group
G = HH * DH            # 512 features per group
SCALING = DH ** -0.5
NEG = -30000.0         # exp(x + NEG) == 0.0 in fp32, no LUT edge cases

F32 = mybir.dt.float32
BF = mybir.dt.bfloat16   # tensor-engine operand dtype (1 cyc/row)


def _mm(ap):
    return ap


def _split_waits(nc):
    """TPB ISA structs hold one sem-wait slot. Tile can emit >1 wait per
    instruction (walrus: 'Too many sync wait commands'); hoist all but the
    last wait onto single-wait NOPs on the same engine, inserted just
    before. Timing is unchanged - the waits would have blocked anyway."""
    k = 0
    for f in nc.m.functions:
        for blk in f.blocks:
            new = []
            for inst in blk.instructions:
                si = inst.sync_info
                w = list(si.on_wait) if si else []
                if len(w) > 1:
                    for wait in w[:-1]:
                        nop = mybir.InstNoOp(name=f"nopw-{k}", ins=[], outs=[])
                        k += 1
                        nop.engine = inst.engine
                        nop.sync_info = mybir.SyncInfo(on_wait=[wait], on_update=[])
                        new.append(nop)
                    inst.sync_info = mybir.SyncInfo(
                        on_wait=[w[-1]], on_update=list(si.on_update)
                    )
                new.append(inst)
            blk.instructions = new
    return nc


def build_nc(split=True, phase='all'):
    nc = bass.Bass()

    xqT = nc.dram_tensor("xqT", [E, T], BF, kind="ExternalInput")
    xkT = nc.dram_tensor("xkT", [E, S], BF, kind="ExternalInput")
    xvT = nc.dram_tensor("xvT", [E, S], BF, kind="ExternalInput")
    wqT = nc.dram_tensor("wqT", [E, G], BF, kind="ExternalInput")
    wkT = nc.dram_tensor("wkT", [E, G], BF, kind="ExternalInput")
    wvT = nc.dram_tensor("wvT", [E, G], BF, kind="ExternalInput")
    woT = nc.dram_tensor("woT", [G, E], BF, kind="ExternalInput")
    mb = nc.dram_tensor("mb", [128, 8], F32, kind="ExternalInput")    # [-30000|0] per s
    bqc = nc.dram_tensor("bqc", [128, 4], F32, kind="ExternalInput")  # bq per j-tile col
    bkc = nc.dram_tensor("bkc", [128, 4], F32, kind="ExternalInput")
    bvr = nc.dram_tensor("bvr", [1, G], BF, kind="ExternalInput")    # bv as row
    out = nc.dram_tensor("out", [T, E], F32, kind="ExternalOutput")

    ET, ST, TT = E // 128, S // 128, T // 128   # 8, 8, 8
    JT = G // 128                               # 4 j-tiles
    NC = 512                                    # moving-operand chunk
    TC = T // NC                                # 2 t-chunks

    with TileContext(nc) as tc:
        with (
            tc.tile_pool(name="const", bufs=1) as pc,
            tc.tile_pool(name="persist", bufs=1) as pp,
            tc.tile_pool(name="xin", bufs=ET) as px,
            tc.tile_pool(name="win", bufs=ET) as pw,
            tc.tile_pool(name="exp", bufs=2 * ST) as pe,
            tc.tile_pool(name="outsb", bufs=3) as po,
            tc.tile_pool(name="small", bufs=4) as psm,
            tc.tile_pool(name="psg", bufs=2, space="PSUM") as ppsg,
            tc.tile_pool(name="pssc", bufs=4, space="PSUM") as ppsc,
            tc.tile_pool(name="psav", bufs=2, space="PSUM") as ppsav,
        ):
            # ---- constants ----
            mb_sb = pc.tile([128, 8], F32, name="mb_sb")
            nc.sync.dma_start(out=mb_sb[:], in_=mb[:])
            bq_sb = pc.tile([128, 4], F32, name="bq_sb")
            nc.sync.dma_start(out=bq_sb[:], in_=bqc[:])
            bk_sb = pc.tile([128, 4], F32, name="bk_sb")
            nc.sync.dma_start(out=bk_sb[:], in_=bkc[:])
            bv_sb = pc.tile([1, G], BF, name="bv_sb")
            nc.sync.dma_start(out=bv_sb[:], in_=bvr[:])
            ones_sb = pc.tile([2, 128], BF, name="ones_sb")
            nc.gpsimd.memset(ones_sb[:], 1.0)

            # ---- persistent activations ----
            qT_sb = [pp.tile([128, T], BF, name=f"qT{r}") for r in range(JT)]
            kT_sb = [pp.tile([128, S], BF, name=f"kT{r}") for r in range(JT)]
            v_sb = [pp.tile([128, HH * (DH + 1)], BF, name=f"v{st}") for st in range(ST)]
            aT_sb = [pp.tile([128, T], BF, name=f"aT{r}") for r in range(JT)]
            woT_sb = [pp.tile([128, E], BF, name=f"woT{r}") for r in range(JT)]

            # ---- q/k projections: out[j,t] = sum_e W.T[e,j] X.T[e,t] (+ bias) ----
            for pi, (xdr, wdr, dst, bias) in enumerate((
                (xqT, wqT, qT_sb, bq_sb),
                (xkT, wkT, kT_sb, bk_sb),
            )):
                xt = [px.tile([128, T], BF, tag=f"xe{pi}", name=f"xe{pi}_{et}") for et in range(ET)]
                wt = [pw.tile([128, G], BF, tag=f"we{pi}", name=f"we{pi}_{et}") for et in range(ET)]
                for et in range(ET):
                    nc.sync.dma_start(out=xt[et][:], in_=xdr[et * 128:(et + 1) * 128, :])
                    nc.sync.dma_start(out=wt[et][:], in_=wdr[et * 128:(et + 1) * 128, :])
                for r in range(JT):
                    for c2 in range(TC):
                        ps = ppsg.tile([128, NC], F32, tag="psg", name="ps_proj")
                        for et in range(ET):
                            nc.tensor.matmul(
                                ps[:],
                                lhsT=_mm(wt[et][:, r * 128:(r + 1) * 128]),
                                rhs=_mm(xt[et][:, c2 * NC:(c2 + 1) * NC]),
                                start=(et == 0), stop=(et == ET - 1),
                            )
                        nc.vector.tensor_scalar_add(
                            dst[r][:, c2 * NC:(c2 + 1) * NC], ps[:], bias[:, r:r + 1]
                        )

            # ---- v projection: v[s,d] = sum_e X.T[e,s] Wv.T[e,d] + bv ----
            xt = [px.tile([128, S], BF, tag="xev", name=f"xve{et}") for et in range(ET)]
            wt = [pw.tile([128, G], BF, tag="wev", name=f"wve{et}") for et in range(ET)]
            for et in range(ET):
                nc.sync.dma_start(out=xt[et][:], in_=xvT[et * 128:(et + 1) * 128, :])
                nc.sync.dma_start(out=wt[et][:], in_=wvT[et * 128:(et + 1) * 128, :])
            for st in range(ST):
                ps = ppsg.tile([128, G], F32, tag="psg", name="ps_v")
                for et in range(ET):
                    nc.tensor.matmul(
                        ps[:],
                        lhsT=_mm(xt[et][:, st * 128:(st + 1) * 128]),
                        rhs=_mm(wt[et][:]),
                        start=(et == 0), stop=False,
                    )
                nc.tensor.matmul(  # += ones[1,128].T @ bv[1,512]
                    ps[:], lhsT=_mm(ones_sb[0:1, :]), rhs=_mm(bv_sb[:]),
                    start=False, stop=True,
                )
                # scatter [128, 8, 64] into 65-col stripes; stripe col 64 <- 1.0
                v3 = v_sb[st][:].rearrange("p (h x) -> p h x", x=DH + 1)
                nc.vector.tensor_copy(
                    v3[:, :, 0:DH], ps[:].rearrange("p (h x) -> p h x", x=DH)
                )
                nc.gpsimd.memset(v3[:, :, DH:DH + 1], 1.0)

            for r in range(JT):
                nc.sync.dma_start(out=woT_sb[r][:], in_=woT[r * 128:(r + 1) * 128, :])

            if phase == 'proj':
                for r in range(JT):
                    ot = po.tile([128, T], F32, tag="otp", name=f"otp{r}")
                    nc.vector.tensor_copy(ot[:], qT_sb[r][:])
                    nc.sync.dma_start(out=out[r * 128:(r + 1) * 128, :], in_=ot[:])
                    ot2 = po.tile([128, T], F32, tag="otp", name=f"otp2{r}")
                    nc.vector.tensor_copy(ot2[:], kT_sb[r][:])
                    nc.sync.dma_start(out=out[512 + r * 128:512 + (r + 1) * 128, :], in_=ot2[:])

            # ---- attention ----
            for c in range(TC if phase == 'all' else 0):
                tsl = slice(c * NC, (c + 1) * NC)
                for hp in range(HH // 2):
                    pair = (2 * hp, 2 * hp + 1)
                    expT = {h: [pe.tile([128, NC], BF, tag="exp", name=f"exp_h{h}_s{st}") for st in range(ST)]
                            for h in pair}
                    for st in range(ST):
                        for h in pair:
                            r, po_ = h // 2, (h % 2) * DH
                            ps_s = ppsc.tile([128, NC], F32, tag="sc", name="ps_s")
                            nc.tensor.matmul(
                                ps_s[:],
                                lhsT=_mm(kT_sb[r][po_:po_ + DH, st * 128:(st + 1) * 128]),
                                rhs=_mm(qT_sb[r][po_:po_ + DH, tsl]),
                                start=True, stop=True,
                            )
                            nc.scalar.activation(
                                expT[h][st][:], ps_s[:],
                                mybir.ActivationFunctionType.Exp,
                                bias=mb_sb[:, st:st + 1], scale=SCALING,
                            )
                    for h in pair:
                        r, po_ = h // 2, (h % 2) * DH
                        ps_o = ppsav.tile([DH + 1, NC], F32, tag="av", name="ps_o")
                        for st in range(ST):
                            nc.tensor.matmul(
                                ps_o[:],
                                lhsT=_mm(v_sb[st][:, h * (DH + 1):(h + 1) * (DH + 1)]),
                                rhs=_mm(expT[h][st][:]),
                                start=(st == 0), stop=(st == ST - 1),
                            )
                        rec = psm.tile([1, NC], F32, tag="rec", name="rec")
                        nc.vector.reciprocal(rec[:], ps_o[DH:DH + 1, :])
                        # broadcast rec across 64 partitions at ~fp32 precision:
                        # hi = bf16(rec), lo = bf16(rec - hi);  ones[2,64].T @ [hi;lo]
                        # sums hi+lo in fp32 PSUM.
                        rhi = psm.tile([1, NC], BF, tag="rhi", name="rhi")
                        nc.vector.tensor_copy(rhi[:], rec[:])
                        rlo = psm.tile([1, NC], BF, tag="rlo", name="rlo")
                        nc.vector.tensor_sub(rlo[:], rec[:], rhi[:])
                        ps_b = ppsc.tile([DH, NC], F32, tag="sc", name="ps_b")
                        nc.tensor.matmul(ps_b[:], lhsT=ones_sb[0:1, 0:DH],
                                         rhs=rhi[:], start=True, stop=False)
                        nc.tensor.matmul(ps_b[:], lhsT=ones_sb[0:1, 0:DH],
                                         rhs=rlo[:], start=False, stop=True)
                        rb = psm.tile([DH, NC], F32, tag="rb", name="rb")
                        nc.vector.tensor_copy(rb[:], ps_b[:])
                        nc.vector.tensor_mul(
                            aT_sb[r][po_:po_ + DH, tsl],
                            ps_o[0:DH, :],
                            rb[:],
                        )
                # ---- output projection for this chunk's t-tiles ----
                for tt in range(c * 4, c * 4 + 4):
                    for oc in range(2):
                        ps_u = ppsg.tile([128, NC], F32, tag="psg", name="ps_u")
                        for r in range(JT):
                            nc.tensor.matmul(
                                ps_u[:],
                                lhsT=_mm(aT_sb[r][:, tt * 128:(tt + 1) * 128]),
                                rhs=_mm(woT_sb[r][:, oc * NC:(oc + 1) * NC]),
                                start=(r == 0), stop=(r == JT - 1),
                            )
                        ot = po.tile([128, NC], F32, tag="ot", name="ot")
                        nc.vector.tensor_copy(ot[:], ps_u[:])
                        nc.sync.dma_start(
                            out=out[tt * 128:(tt + 1) * 128, oc * NC:(oc + 1) * NC],
                            in_=ot[:],
                        )
    return _split_waits(nc) if split else nc


_NC_CACHE = None


def _get_nc():
    global _NC_CACHE
    if _NC_CACHE is None:
        _NC_CACHE = build_nc()
    return _NC_CACHE


def make_in_maps(query, key, value, key_padding_mask, local_mask,
                 Wq, bq, Wk, bk, Wv, bv, Wo, bo):
    import ml_dtypes
    f = np.float32
    bf = ml_dtypes.bfloat16
    in_maps = []
    for c in range(8):
        b, g = c // 2, c % 2
        gs = slice(g * G, (g + 1) * G)
        mask = (key_padding_mask if g == 0 else local_mask)[b]
        mbias = np.where(mask, NEG, 0.0).astype(f).reshape(8, 128).T  # [128, 8]
        in_maps.append({
            "xqT": np.ascontiguousarray(query[b].T, dtype=bf),
            "xkT": np.ascontiguousarray(key[b].T, dtype=bf),
            "xvT": np.ascontiguousarray(value[b].T, dtype=bf),
            "wqT": np.ascontiguousarray(Wq[gs, :].T, dtype=bf),
            "wkT": np.ascontiguousarray(Wk[gs, :].T, dtype=bf),
            "wvT": np.ascontiguousarray(Wv[gs, :].T, dtype=bf),
            "woT": np.ascontiguousarray(Wo[:, gs].T, dtype=bf),
            "mb": np.ascontiguousarray(mbias),
            "bqc": np.ascontiguousarray(bq[gs].astype(f).reshape(4, 128).T),
            "bkc": np.ascontiguousarray(bk[gs].astype(f).reshape(4, 128).T),
            "bvr": np.ascontiguousarray(bv[gs].astype(bf).reshape(1, G)),
        })
    return in_maps


def kernel(query, key, value, key_padding_mask, local_mask,
           Wq, bq, Wk, bk, Wv, bv, Wo, bo, _trace=False, _tmpdir=None):
    from concourse.bass_utils import run_bass_kernel_spmd

    nc = _get_nc()
    in_maps = make_in_maps(query, key, value, key_padding_mask, local_mask,
                           Wq, bq, Wk, bk, Wv, bv, Wo, bo)
    try:
        res = run_bass_kernel_spmd(nc, in_maps, list(range(8)),
                                   trace=_trace, tmpdir=_tmpdir)
    except Exception:
        # transient device/transport failures have been observed on the
        # axon path; one fresh attempt is cheap relative to a hard fail
        res = run_bass_kernel_spmd(nc, in_maps, list(range(8)),
                                   trace=_trace, tmpdir=_tmpdir)
    outs = [np.asarray(r["out"]) for r in res.results]
    full = np.stack([outs[2 * b] + outs[2 * b + 1] for b in range(B)])
    full += np.asarray(bo, dtype=np.float32)
    if _trace:
        kernel._last_exec_time_ns = res.exec_time_ns
        kernel._last_profile = res.profile_json
    return full.astype(np.float32)



# revision 3
# speedup vs baseline: 1.2788x; 1.2788x over previous
"""Cross multihead attention (global/local masked head groups) on 8 trn2 cores.

Sharding: core c -> (batch b = c//2, head-group g = c%2).
  g=0: heads 0-7  masked by key_padding_mask[b]
  g=1: heads 8-15 masked by local_mask[b]
Each core computes its group's partial output (attn_out_g @ Wo[:, gs].T)
of shape [T, E]; the host sums the two partials per batch and adds bo.

Key structure (vs a straightforward port):
  - Masked keys contribute nothing (exp -> 0), so the host gathers the
    ~50% unmasked key/value columns into an SQ=640-padded buffer;
    k/v projections, QK^T and AV all shrink accordingly.  Padding
    columns keep a -30000 exp bias so they vanish from the softmax.
  - scoresT orientation [s, t]: mask bias is per-partition for the exp.
  - AV is computed as out[t, d] = expT.T @ [v | ones]: the moving operand
    is only 65 wide (cost ~ moving free size), and the ones column gives
    the softmax denominator per t-partition, so normalization is a
    per-partition scalar multiply (no partition-broadcast needed).
  - A PE transpose (with an identity moving operand) flips normalized
    [t, j] pairs back to [j, t] for the output projection.
  - All big DRAM loads are single batched DMAs (HWDGE serializes at
    ~625ns per dma_start).
  - PE order: k-proj r0..3, then (q-proj r, QK r) interleaved so exp
    (Act engine) overlaps remaining projections, then v-proj, then
    AV/normalize/transpose per head-pair, with the output projection
    interleaved into the last head-pair's loop.
"""

import os
import sys

sys.path.insert(0, "/opt/trn_rl_repo")

import numpy as np

import concourse.bass as bass
import concourse.mybir as mybir
from concourse.tile import TileContext

B, T, S, E, H = 4, 1024, 1024, 1024, 16
DH = E // H            # 64
HH = H // 2            # 8 heads per group
G = HH * DH            # 512 features per group
SCALING = DH ** -0.5
NEG = -30000.0         # exp(x + NEG) == 0.0 in fp32, no LUT edge cases

F32 = mybir.dt.float32
BF = mybir.dt.bfloat16

ET = E // 128          # 8 contraction tiles
JT = G // 128          # 4 j-tiles (head pairs)


def _split_waits(nc):
    """TPB ISA structs hold one sem-wait slot. Tile can emit >1 wait per
    instruction (walrus: 'Too many sync wait commands'); hoist all but the
    last wait onto single-wait NOPs on the same engine, inserted just
    before. Timing is unchanged - the waits would have blocked anyway."""
    k = 0
    for f in nc.m.functions:
        for blk in f.blocks:
            new = []
            for inst in blk.instructions:
                si = inst.sync_info
                w = list(si.on_wait) if si else []
                if len(w) > 1:
                    for wait in w[:-1]:
                        nop = mybir.InstNoOp(name=f"nopw-{k}", ins=[], outs=[])
                        k += 1
                        nop.engine = inst.engine
                        nop.sync_info = mybir.SyncInfo(on_wait=[wait], on_update=[])
                        new.append(nop)
                    inst.sync_info = mybir.SyncInfo(
                        on_wait=[w[-1]], on_update=list(si.on_update)
                    )
                new.append(inst)
            blk.instructions = new
    return nc


def build_nc(st_tiles=5, split=True):
    ST = st_tiles          # number of 128-wide s tiles after compression
    SQ = ST * 128          # padded compressed key count

    nc = bass.Bass()

    xqT = nc.dram_tensor("xqT", [E, T], BF, kind="ExternalInput")
    xkT = nc.dram_tensor("xkT", [E, SQ], BF, kind="ExternalInput")
    xvT = nc.dram_tensor("xvT", [E, SQ], BF, kind="ExternalInput")
    wqT = nc.dram_tensor("wqT", [E, G], BF, kind="ExternalInput")
    wkT = nc.dram_tensor("wkT", [E, G], BF, kind="ExternalInput")
    wvT = nc.dram_tensor("wvT", [E, G], BF, kind="ExternalInput")
    woT = nc.dram_tensor("woT", [G, E], BF, kind="ExternalInput")
    mb = nc.dram_tensor("mb", [128, ST], F32, kind="ExternalInput")
    bqc = nc.dram_tensor("bqc", [128, JT], F32, kind="ExternalInput")
    bkc = nc.dram_tensor("bkc", [128, JT], F32, kind="ExternalInput")
    bvr = nc.dram_tensor("bvr", [1, G], BF, kind="ExternalInput")
    idn = nc.dram_tensor("idn", [128, 128], BF, kind="ExternalInput")
    out = nc.dram_tensor("out", [T, E], F32, kind="ExternalOutput")

    TT = T // 128          # 8 t tiles

    with TileContext(nc) as tc:
        with (
            tc.tile_pool(name="const", bufs=1) as pc,
            tc.tile_pool(name="persist", bufs=1) as pp,
            tc.tile_pool(name="exp", bufs=HH * ST) as pe,
            tc.tile_pool(name="small", bufs=4) as psm,
            tc.tile_pool(name="outsb", bufs=2) as po,
            tc.tile_pool(name="ps", bufs=2, space="PSUM") as pps,
        ):
            # ---- constants ----
            mb_sb = pc.tile([128, ST], F32, name="mb_sb")
            nc.sync.dma_start(out=mb_sb[:], in_=mb[:])
            bq_sb = pc.tile([128, JT], F32, name="bq_sb")
            nc.sync.dma_start(out=bq_sb[:], in_=bqc[:])
            bk_sb = pc.tile([128, JT], F32, name="bk_sb")
            nc.sync.dma_start(out=bk_sb[:], in_=bkc[:])
            bv_sb = pc.tile([1, G], BF, name="bv_sb")
            nc.sync.dma_start(out=bv_sb[:], in_=bvr[:])
            id_sb = pc.tile([128, 128], BF, name="id_sb")
            nc.sync.dma_start(out=id_sb[:], in_=idn[:])
            ones_sb = pc.tile([1, 128], BF, name="ones_sb")
            nc.gpsimd.memset(ones_sb[:], 1.0)

            # ---- batched input loads (one DMA per tensor) ----
            xk_sb = pc.tile([128, ET * SQ], BF, name="xk_sb")
            nc.sync.dma_start(
                out=xk_sb[:].rearrange("p (e t) -> p e t", e=ET),
                in_=xkT[:].rearrange("(e p) t -> p e t", p=128),
            )
            wk_sb = pc.tile([128, ET * G], BF, name="wk_sb")
            nc.sync.dma_start(
                out=wk_sb[:].rearrange("p (e t) -> p e t", e=ET),
                in_=wkT[:].rearrange("(e p) t -> p e t", p=128),
            )
            wq_sb = pc.tile([128, ET * G], BF, name="wq_sb")
            nc.sync.dma_start(
                out=wq_sb[:].rearrange("p (e t) -> p e t", e=ET),
                in_=wqT[:].rearrange("(e p) t -> p e t", p=128),
            )
            xq_sb = pc.tile([128, ET * T], BF, name="xq_sb")
            nc.sync.dma_start(
                out=xq_sb[:].rearrange("p (e t) -> p e t", e=ET),
                in_=xqT[:].rearrange("(e p) t -> p e t", p=128),
            )
            wv_sb = pc.tile([128, ET * G], BF, name="wv_sb")
            nc.sync.dma_start(
                out=wv_sb[:].rearrange("p (e t) -> p e t", e=ET),
                in_=wvT[:].rearrange("(e p) t -> p e t", p=128),
            )
            xv_sb = pc.tile([128, ET * SQ], BF, name="xv_sb")
            nc.sync.dma_start(
                out=xv_sb[:].rearrange("p (e t) -> p e t", e=ET),
                in_=xvT[:].rearrange("(e p) t -> p e t", p=128),
            )
            wo_sb = pc.tile([128, JT * E], BF, name="wo_sb")
            nc.sync.dma_start(
                out=wo_sb[:].rearrange("p (r t) -> p r t", r=JT),
                in_=woT[:].rearrange("(r p) t -> p r t", p=128),
            )
            xk3 = xk_sb[:].rearrange("p (e t) -> p e t", e=ET)
            wk3 = wk_sb[:].rearrange("p (e t) -> p e t", e=ET)
            xq3 = xq_sb[:].rearrange("p (e t) -> p e t", e=ET)
            wq3 = wq_sb[:].rearrange("p (e t) -> p e t", e=ET)
            xv3 = xv_sb[:].rearrange("p (e t) -> p e t", e=ET)
            wv3 = wv_sb[:].rearrange("p (e t) -> p e t", e=ET)
            wo3 = wo_sb[:].rearrange("p (r t) -> p r t", r=JT)

            # ---- persistent activations ----
            qT_sb = [pp.tile([128, T], BF, name=f"qT{r}") for r in range(JT)]
            kT_sb = [pp.tile([128, SQ], BF, name=f"kT{r}") for r in range(JT)]
            v_sb = [pp.tile([128, HH * (DH + 1)], BF, name=f"v{st}") for st in range(ST)]
            aT_sb = [pp.tile([128, T], BF, name=f"aT{r}") for r in range(JT)]

            # ---- k projection: kT[j, s] = sum_e Wk.T[e, j] Xk.T[e, s] + bk ----
            for r in range(JT):
                ps = pps.tile([128, 1024], F32, tag="qk", name="ps_k")
                for lo, hi in ((0, 512), (512, SQ)):
                    if lo >= SQ:
                        continue
                    for et in range(ET):
                        nc.tensor.matmul(
                            ps[:, lo:hi],
                            lhsT=wk3[:, et, r * 128:(r + 1) * 128],
                            rhs=xk3[:, et, lo:hi],
                            start=(et == 0), stop=(et == ET - 1),
                        )
                nc.vector.tensor_scalar_add(
                    kT_sb[r][:], ps[:, 0:SQ], bk_sb[:, r:r + 1]
                )

            # ---- q projection + QK^T + exp, interleaved per j-tile ----
            expT = [None] * (HH * ST)
            for r in range(JT):
                ps = pps.tile([128, 1024], F32, tag="qk", name="ps_q")
                for lo, hi in ((0, 512), (512, 1024)):
                    for et in range(ET):
                        nc.tensor.matmul(
                            ps[:, lo:hi],
                            lhsT=wq3[:, et, r * 128:(r + 1) * 128],
                            rhs=xq3[:, et, lo:hi],
                            start=(et == 0), stop=(et == ET - 1),
                        )
                nc.vector.tensor_scalar_add(
                    qT_sb[r][:], ps[:], bq_sb[:, r:r + 1]
                )
                # scoresT[s, t] for the two heads of this j-tile
                for hl in range(2):
                    h = 2 * r + hl
                    po_ = hl * DH
                    for st in range(ST):
                        ps_s = pps.tile([128, 1024], F32, tag="qk", name="ps_s")
                        for lo, hi in ((0, 512), (512, 1024)):
                            nc.tensor.matmul(
                                ps_s[:, lo:hi],
                                lhsT=kT_sb[r][po_:po_ + DH, st * 128:(st + 1) * 128],
                                rhs=qT_sb[r][po_:po_ + DH, lo:hi],
                                start=True, stop=True,
                            )
                        ex = pe.tile([128, T], BF, tag="exp", name=f"exp{h}_{st}")
                        expT[h * ST + st] = ex
                        nc.scalar.activation(
                            ex[:], ps_s[:],
                            mybir.ActivationFunctionType.Exp,
                            bias=mb_sb[:, st:st + 1], scale=SCALING,
                        )

            # ---- v projection: v[s, d] = sum_e Xv.T[e, s] Wv.T[e, d] + bv ----
            for st in range(ST):
                ps = pps.tile([128, 1024], F32, tag="qk", name="ps_v")
                for et in range(ET):
                    nc.tensor.matmul(
                        ps[:, 0:G],
                        lhsT=xv3[:, et, st * 128:(st + 1) * 128],
                        rhs=wv3[:, et, :],
                        start=(et == 0), stop=False,
                    )
                nc.tensor.matmul(  # += ones[1,128].T @ bv[1,512]
                    ps[:, 0:G], lhsT=ones_sb[:], rhs=bv_sb[:],
                    start=False, stop=True,
                )
                v3 = v_sb[st][:].rearrange("p (h x) -> p h x", x=DH + 1)
                nc.vector.tensor_copy(
                    v3[:, :, 0:DH], ps[:, 0:G].rearrange("p (h x) -> p h x", x=DH)
                )
                nc.gpsimd.memset(v3[:, :, DH:DH + 1], 1.0)

            # ---- attention: AV + normalize + transpose; outproj on last hp ----
            for hp in range(JT):
                for tt in range(TT):
                    pav = pps.tile([128, 1024], F32, tag="avp", name="pav")
                    for i in range(2):
                        h = 2 * hp + i
                        base = i * 512
                        for st in range(ST):
                            nc.tensor.matmul(
                                pav[:, base:base + DH + 1],
                                lhsT=expT[h * ST + st][:, tt * 128:(tt + 1) * 128],
                                rhs=v_sb[st][:, h * (DH + 1):(h + 1) * (DH + 1)],
                                start=(st == 0), stop=(st == ST - 1),
                            )
                    rec = psm.tile([128, 2], F32, tag="rec", name="rec")
                    nc.vector.reciprocal(
                        rec[:],
                        pav[:].rearrange("p (b x) -> p b x", b=2)[:, :, DH:DH + 1],
                    )
                    op = psm.tile([128, 128], BF, tag="op", name="op")
                    nc.vector.tensor_scalar_mul(
                        op[:, 0:DH], pav[:, 0:DH], rec[:, 0:1]
                    )
                    nc.scalar.activation(
                        op[:, DH:2 * DH], pav[:, 512:512 + DH],
                        mybir.ActivationFunctionType.Copy,
                        scale=rec[:, 1:2],
                    )
                    tr = pps.tile([128, 1024], BF, tag="qk", name="tr")
                    nc.tensor.transpose(tr[:, 0:128], op[:], id_sb[:])
                    nc.vector.tensor_copy(
                        aT_sb[hp][:, tt * 128:(tt + 1) * 128], tr[:, 0:128]
                    )
                    if hp == JT - 1 and tt % 2 == 1:
                        for t2 in (tt - 1, tt):
                            osb = po.tile([128, E], F32, tag="osb", name="osb")
                            for oc in range(2):
                                pu = pps.tile([128, 1024], F32, tag="qk", name="pu")
                                for r in range(JT):
                                    nc.tensor.matmul(
                                        pu[:, 0:512],
                                        lhsT=aT_sb[r][:, t2 * 128:(t2 + 1) * 128],
                                        rhs=wo3[:, r, oc * 512:(oc + 1) * 512],
                                        start=(r == 0), stop=(r == JT - 1),
                                    )
                                if oc == 0:
                                    nc.vector.tensor_copy(
                                        osb[:, 0:512], pu[:, 0:512]
                                    )
                                else:
                                    nc.scalar.copy(
                                        osb[:, 512:1024], pu[:, 0:512]
                                    )
                            nc.sync.dma_start(
                                out=out[t2 * 128:(t2 + 1) * 128, :], in_=osb[:]
                            )
    return _split_waits(nc) if split else nc


_NC_CACHE = {}


def _get_nc(st_tiles=5):
    if st_tiles not in _NC_CACHE:
        _NC_CACHE[st_tiles] = build_nc(st_tiles)
    return _NC_CACHE[st_tiles]


def make_in_maps(query, key, value, key_padding_mask, local_mask,
                 Wq, bq, Wk, bk, Wv, bv, Wo, bo, st_tiles=5):
    import ml_dtypes
    f = np.float32
    bf = ml_dtypes.bfloat16
    SQ = st_tiles * 128
    ident = np.eye(128, dtype=bf)
    in_maps = []
    for c in range(8):
        b, g = c // 2, c % 2
        gs = slice(g * G, (g + 1) * G)
        mask = np.asarray((key_padding_mask if g == 0 else local_mask)[b])
        sel = np.flatnonzero(~mask)
        ns = sel.size
        assert ns <= SQ, (ns, SQ)
        xk = np.zeros((E, SQ), dtype=bf)
        xk[:, :ns] = np.asarray(key[b]).T[:, sel]
        xv = np.zeros((E, SQ), dtype=bf)
        xv[:, :ns] = np.asarray(value[b]).T[:, sel]
        mbias = np.full(SQ, NEG, f)
        mbias[:ns] = 0.0
        in_maps.append({
            "xqT": np.ascontiguousarray(np.asarray(query[b]).T, dtype=bf),
            "xkT": xk,
            "xvT": xv,
            "wqT": np.ascontiguousarray(np.asarray(Wq)[gs, :].T, dtype=bf),
            "wkT": np.ascontiguousarray(np.asarray(Wk)[gs, :].T, dtype=bf),
            "wvT": np.ascontiguousarray(np.asarray(Wv)[gs, :].T, dtype=bf),
            "woT": np.ascontiguousarray(np.asarray(Wo)[:, gs].T, dtype=bf),
            "mb": np.ascontiguousarray(mbias.reshape(st_tiles, 128).T),
            "bqc": np.ascontiguousarray(np.asarray(bq)[gs].astype(f).reshape(JT, 128).T),
            "bkc": np.ascontiguousarray(np.asarray(bk)[gs].astype(f).reshape(JT, 128).T),
            "bvr": np.ascontiguousarray(np.asarray(bv)[gs].astype(bf).reshape(1, G)),
            "idn": ident,
        })
    return in_maps


def _needed_st_tiles(key_padding_mask, local_mask):
    worst = 0
    for c in range(8):
        b, g = c // 2, c % 2
        mask = np.asarray((key_padding_mask if g == 0 else local_mask)[b])
        worst = max(worst, int((~mask).sum()))
    return max(1, -(-worst // 128))


def kernel(query, key, value, key_padding_mask, local_mask,
           Wq, bq, Wk, bk, Wv, bv, Wo, bo, _trace=False, _tmpdir=None):
    from concourse.bass_utils import run_bass_kernel_spmd

    st = min(max(_needed_st_tiles(key_padding_mask, local_mask), 5), 8)
    nc = _get_nc(st)
    in_maps = make_in_maps(query, key, value, key_padding_mask, local_mask,
                           Wq, bq, Wk, bk, Wv, bv, Wo, bo, st_tiles=st)
    try:
        res = run_bass_kernel_spmd(nc, in_maps, list(range(8)),
                                   trace=_trace, tmpdir=_tmpdir)
    except Exception:
        # transient device/transport failures have been observed on the
        # axon path; one fresh attempt is cheap relative to a hard fail
        res = run_bass_kernel_spmd(nc, in_maps, list(range(8)),
                                   trace=_trace, tmpdir=_tmpdir)
    outs = [np.asarray(r["out"]) for r in res.results]
    full = np.stack([outs[2 * b] + outs[2 * b + 1] for b in range(B)])
    full += np.asarray(bo, dtype=np.float32)
    if _trace:
        kernel._last_exec_time_ns = res.exec_time_ns
        kernel._last_profile = res.profile_json
    return full.astype(np.float32)


# revision 6
# speedup vs baseline: 1.5259x; 1.1932x over previous
"""Cross multihead attention (global/local masked head groups) on 8 trn2 cores.

Sharding: core c -> (batch b = c//2, head-group g = c%2).
  g=0: heads 0-7  masked by key_padding_mask[b]
  g=1: heads 8-15 masked by local_mask[b]
Each core computes its group's partial output (attn_out_g @ Wo[:, gs].T)
of shape [T, E]; the host sums the two partials per batch and adds bo.

Key structure:
  - Masked keys contribute nothing (exp -> 0), so the host gathers the
    ~50% unmasked key/value columns into an SQ=640-padded buffer;
    k/v projections, QK^T and AV all shrink accordingly.  Padding
    columns keep a -30000 exp bias so they vanish from the softmax.
  - scoresT orientation [s, t]: mask bias is per-partition for the exp.
  - AV is computed as out[t, d] = expT.T @ [v | ones]: the moving operand
    is only 65 wide (PE cost ~ moving free size), and the ones column
    gives the softmax denominator per t-partition, so normalization is
    a per-partition scalar multiply (no partition broadcast needed).
  - A PE transpose (identity moving operand) flips normalized [t, j]
    head pairs back to [j, t] for the output projection.
  - All big DRAM loads are single batched DMAs (HWDGE serializes at
    ~625ns per dma_start).
  - Software-pipelined emission: the PE stream interleaves "A units"
    (q-proj + QK + exp, which pace at the Act engine's exp speed via
    PSUM buffer rotation) with "B units" (v-proj, AV chains, output
    projection) so the PE never idles while Act catches up.
"""

import os
import sys

sys.path.insert(0, "/opt/trn_rl_repo")

import numpy as np

import concourse.bass as bass
import concourse.mybir as mybir
from concourse.tile import TileContext

B, T, S, E, H = 4, 1024, 1024, 1024, 16
DH = E // H            # 64
HH = H // 2            # 8 heads per group
G = HH * DH            # 512 features per group
SCALING = DH ** -0.5
NEG = -30000.0         # exp(x + NEG) == 0.0 in fp32, no LUT edge cases

F32 = mybir.dt.float32
BF = mybir.dt.bfloat16

ET = E // 128          # 8 contraction tiles
JT = G // 128          # 4 j-tiles (head pairs)
TT = T // 128          # 8 t tiles


def _split_waits(nc):
    """TPB ISA structs hold one sem-wait slot. Tile can emit >1 wait per
    instruction (walrus: 'Too many sync wait commands'); hoist all but the
    last wait onto single-wait NOPs on the same engine, inserted just
    before. Timing is unchanged - the waits would have blocked anyway."""
    k = 0
    for f in nc.m.functions:
        for blk in f.blocks:
            new = []
            for inst in blk.instructions:
                si = inst.sync_info
                w = list(si.on_wait) if si else []
                if len(w) > 1:
                    for wait in w[:-1]:
                        nop = mybir.InstNoOp(name=f"nopw-{k}", ins=[], outs=[])
                        k += 1
                        nop.engine = inst.engine
                        nop.sync_info = mybir.SyncInfo(on_wait=[wait], on_update=[])
                        new.append(nop)
                    inst.sync_info = mybir.SyncInfo(
                        on_wait=[w[-1]], on_update=list(si.on_update)
                    )
                new.append(inst)
            blk.instructions = new
    return nc


def build_nc(st_tiles=5, split=True, off=11):
    ST = st_tiles          # number of 128-wide s tiles after compression
    SQ = ST * 128          # padded compressed key count

    nc = bass.Bass()

    xqT = nc.dram_tensor("xqT", [E, T], BF, kind="ExternalInput")
    xkT = nc.dram_tensor("xkT", [E, SQ], BF, kind="ExternalInput")
    xvT = nc.dram_tensor("xvT", [E, SQ], BF, kind="ExternalInput")
    wqT = nc.dram_tensor("wqT", [E, G], BF, kind="ExternalInput")
    wkT = nc.dram_tensor("wkT", [E, G], BF, kind="ExternalInput")
    wvT = nc.dram_tensor("wvT", [E, G], BF, kind="ExternalInput")
    woT = nc.dram_tensor("woT", [G, E], BF, kind="ExternalInput")
    # packed f32 consts: bq | bk | mb  -> [128, 4 + 4 + ST]
    cf = nc.dram_tensor("cf", [128, 2 * JT + ST], F32, kind="ExternalInput")
    bvr = nc.dram_tensor("bvr", [1, G], BF, kind="ExternalInput")
    idn = nc.dram_tensor("idn", [128, 128], BF, kind="ExternalInput")
    out = nc.dram_tensor("out", [T, E], F32, kind="ExternalOutput")

    with TileContext(nc) as tc:
        with (
            tc.tile_pool(name="const", bufs=1) as pc,
            tc.tile_pool(name="persist", bufs=1) as pp,
            tc.tile_pool(name="exp", bufs=HH * ST) as pe,
            tc.tile_pool(name="small", bufs=4) as psm,
            tc.tile_pool(name="outsb", bufs=2) as po,
            tc.tile_pool(name="ps", bufs=2, space="PSUM") as pps,
        ):
            # ---- staged loads, startup-latency ordered ----
            cf_sb = pc.tile([128, 2 * JT + ST], F32, name="cf_sb")
            nc.sync.dma_start(out=cf_sb[:], in_=cf[:])
            bq_sb = cf_sb[:, 0:JT]
            bk_sb = cf_sb[:, JT:2 * JT]
            mb_sb = cf_sb[:, 2 * JT:]

            wk_sb = pc.tile([128, ET * G], BF, name="wk_sb")
            nc.sync.dma_start(
                out=wk_sb[:].rearrange("p (e t) -> p e t", e=ET),
                in_=wkT[:].rearrange("(e p) t -> p e t", p=128),
            )
            xk_sb = pc.tile([128, ET * SQ], BF, name="xk_sb")
            nc.sync.dma_start(
                out=xk_sb[:].rearrange("p (e t) -> p e t", e=ET),
                in_=xkT[:].rearrange("(e p) t -> p e t", p=128),
            )
            wq_sb = pc.tile([128, ET * G], BF, name="wq_sb")
            nc.sync.dma_start(
                out=wq_sb[:].rearrange("p (e t) -> p e t", e=ET),
                in_=wqT[:].rearrange("(e p) t -> p e t", p=128),
            )
            xq_sb = pc.tile([128, ET * T], BF, name="xq_sb")
            nc.sync.dma_start(
                out=xq_sb[:].rearrange("p (e t) -> p e t", e=ET),
                in_=xqT[:].rearrange("(e p) t -> p e t", p=128),
            )
            bv_sb = pc.tile([1, G], BF, name="bv_sb")
            nc.sync.dma_start(out=bv_sb[:], in_=bvr[:])
            id_sb = pc.tile([128, 128], BF, name="id_sb")
            nc.sync.dma_start(out=id_sb[:], in_=idn[:])
            wv_sb = pc.tile([128, ET * G], BF, name="wv_sb")
            nc.sync.dma_start(
                out=wv_sb[:].rearrange("p (e t) -> p e t", e=ET),
                in_=wvT[:].rearrange("(e p) t -> p e t", p=128),
            )
            xv_sb = pc.tile([128, ET * SQ], BF, name="xv_sb")
            nc.sync.dma_start(
                out=xv_sb[:].rearrange("p (e t) -> p e t", e=ET),
                in_=xvT[:].rearrange("(e p) t -> p e t", p=128),
            )
            wo_sb = pc.tile([128, JT * E], BF, name="wo_sb")
            nc.sync.dma_start(
                out=wo_sb[:].rearrange("p (r t) -> p r t", r=JT),
                in_=woT[:].rearrange("(r p) t -> p r t", p=128),
            )
            ones_sb = pc.tile([1, 128], BF, name="ones_sb")
            nc.gpsimd.memset(ones_sb[:], 1.0)

            xk3 = xk_sb[:].rearrange("p (e t) -> p e t", e=ET)
            wk3 = wk_sb[:].rearrange("p (e t) -> p e t", e=ET)
            xq3 = xq_sb[:].rearrange("p (e t) -> p e t", e=ET)
            wq3 = wq_sb[:].rearrange("p (e t) -> p e t", e=ET)
            xv3 = xv_sb[:].rearrange("p (e t) -> p e t", e=ET)
            wv3 = wv_sb[:].rearrange("p (e t) -> p e t", e=ET)
            wo3 = wo_sb[:].rearrange("p (r t) -> p r t", r=JT)

            # ---- persistent activations ----
            qT_sb = [pp.tile([128, T], BF, name=f"qT{r}") for r in range(JT)]
            kT_sb = [pp.tile([128, SQ], BF, name=f"kT{r}") for r in range(JT)]
            v_sb = [pp.tile([128, HH * (DH + 1)], BF, name=f"v{st}") for st in range(ST)]
            aT_sb = [pp.tile([128, T], BF, name=f"aT{r}") for r in range(JT)]
            expT = [None] * (HH * ST)

            # ---- k projection (pure PE warmup while other DMAs land) ----
            for r in range(JT):
                ps = pps.tile([128, 1024], F32, tag="qk", name="ps_k")
                for lo, hi in ((0, 512), (512, SQ)):
                    if lo >= SQ:
                        continue
                    for et in range(ET):
                        nc.tensor.matmul(
                            ps[:, lo:hi],
                            lhsT=wk3[:, et, r * 128:(r + 1) * 128],
                            rhs=xk3[:, et, lo:hi],
                            start=(et == 0), stop=(et == ET - 1),
                        )
                nc.vector.tensor_scalar_add(
                    kT_sb[r][:], ps[:, 0:SQ], bk_sb[:, r:r + 1]
                )

            # ---- unit emitters ----
            def emit_q(r):
                ps = pps.tile([128, 1024], F32, tag="qk", name="ps_q")
                for lo, hi in ((0, 512), (512, 1024)):
                    for et in range(ET):
                        nc.tensor.matmul(
                            ps[:, lo:hi],
                            lhsT=wq3[:, et, r * 128:(r + 1) * 128],
                            rhs=xq3[:, et, lo:hi],
                            start=(et == 0), stop=(et == ET - 1),
                        )
                nc.vector.tensor_scalar_add(qT_sb[r][:], ps[:], bq_sb[:, r:r + 1])

            def emit_qk(r, hl, st):
                h = 2 * r + hl
                po_ = hl * DH
                ps_s = pps.tile([128, 1024], F32, tag="qk", name="ps_s")
                for lo, hi in ((0, 512), (512, 1024)):
                    nc.tensor.matmul(
                        ps_s[:, lo:hi],
                        lhsT=kT_sb[r][po_:po_ + DH, st * 128:(st + 1) * 128],
                        rhs=qT_sb[r][po_:po_ + DH, lo:hi],
                        start=True, stop=True,
                    )
                ex = pe.tile([128, T], BF, tag="exp", name=f"exp{h}_{st}")
                expT[h * ST + st] = ex
                nc.scalar.activation(
                    ex[:], ps_s[:],
                    mybir.ActivationFunctionType.Exp,
                    bias=mb_sb[:, st:st + 1], scale=SCALING,
                )

            def emit_v(st):
                ps = pps.tile([128, 1024], F32, tag="qk", name="ps_v")
                for et in range(ET):
                    nc.tensor.matmul(
                        ps[:, 0:G],
                        lhsT=xv3[:, et, st * 128:(st + 1) * 128],
                        rhs=wv3[:, et, :],
                        start=(et == 0), stop=False,
                    )
                nc.tensor.matmul(  # += ones[1,128].T @ bv[1,512]
                    ps[:, 0:G], lhsT=ones_sb[:], rhs=bv_sb[:],
                    start=False, stop=True,
                )
                v3 = v_sb[st][:].rearrange("p (h x) -> p h x", x=DH + 1)
                nc.vector.tensor_copy(
                    v3[:, :, 0:DH], ps[:, 0:G].rearrange("p (h x) -> p h x", x=DH)
                )
                nc.gpsimd.memset(v3[:, :, DH:DH + 1], 1.0)

            pav_cur = [None]

            def emit_av(hp, tt, i):
                h = 2 * hp + i
                if i == 0:
                    pav_cur[0] = pps.tile([128, 1024], F32, tag="avp",
                                          name="pav", bufs=1)
                pav = pav_cur[0]
                base = i * 512
                for st in range(ST):
                    nc.tensor.matmul(
                        pav[:, base:base + DH + 1],
                        lhsT=expT[h * ST + st][:, tt * 128:(tt + 1) * 128],
                        rhs=v_sb[st][:, h * (DH + 1):(h + 1) * (DH + 1)],
                        start=(st == 0), stop=(st == ST - 1),
                    )
                if i == 1:
                    rec = psm.tile([128, 2], F32, tag="rec", name="rec")
                    nc.vector.reciprocal(
                        rec[:],
                        pav[:].rearrange("p (b x) -> p b x", b=2)[:, :, DH:DH + 1],
                    )
                    op = psm.tile([128, 128], BF, tag="op", name="op")
                    nc.vector.tensor_scalar_mul(
                        op[:, 0:DH], pav[:, 0:DH], rec[:, 0:1]
                    )
                    nc.vector.tensor_scalar_mul(
                        op[:, DH:2 * DH], pav[:, 512:512 + DH], rec[:, 1:2]
                    )
                    tr = pps.tile([128, 128], BF, tag="tr", name="tr")
                    nc.tensor.transpose(tr[:], op[:], id_sb[:])
                    nc.vector.tensor_copy(
                        aT_sb[hp][:, tt * 128:(tt + 1) * 128], tr[:]
                    )

            def emit_op(t2):
                osb = po.tile([128, E], F32, tag="osb", name="osb")
                for oc in range(2):
                    pu = pps.tile([128, 1024], F32, tag="qk", name="pu")
                    for r in range(JT):
                        nc.tensor.matmul(
                            pu[:, 0:512],
                            lhsT=aT_sb[r][:, t2 * 128:(t2 + 1) * 128],
                            rhs=wo3[:, r, oc * 512:(oc + 1) * 512],
                            start=(r == 0), stop=(r == JT - 1),
                        )
                    if oc == 0:
                        nc.vector.tensor_copy(osb[:, 0:512], pu[:, 0:512])
                    else:
                        nc.scalar.copy(osb[:, 512:1024], pu[:, 0:512])
                nc.sync.dma_start(
                    out=out[t2 * 128:(t2 + 1) * 128, :], in_=osb[:]
                )

            # ---- interleaved A/B emission ----
            a_list = []
            for r in range(JT):
                a_list.append(("q", r))
                for hl in range(2):
                    for st in range(ST):
                        a_list.append(("qk", r, hl, st))
            b_list = [("v", st) for st in range(ST)]
            for hp in range(JT):
                for tt in range(TT):
                    b_list.append(("av", hp, tt, 0))
                    b_list.append(("av", hp, tt, 1))
                    if hp == JT - 1 and tt >= 1:
                        b_list.append(("op", tt - 1))
            b_list.append(("op", TT - 1))

            def emit(u):
                kind = u[0]
                if kind == "q":
                    emit_q(u[1])
                elif kind == "qk":
                    emit_qk(u[1], u[2], u[3])
                elif kind == "v":
                    emit_v(u[1])
                elif kind == "av":
                    emit_av(u[1], u[2], u[3])
                elif kind == "op":
                    emit_op(u[1])

            def a_idx(r, hl, st):
                return r * (2 * ST + 1) + 1 + hl * ST + st

            def prereq(u):
                if u[0] == "av":
                    _, hp, tt, i = u
                    return a_idx(hp, i, ST - 1)
                if u[0] == "op":
                    return a_idx(JT - 1, 1, ST - 1)
                return 0

            nA, nB = len(a_list), len(b_list)
            ib = 0
            for i, u in enumerate(a_list):
                emit(u)
                if i + 1 > off:
                    tgt = ((i + 1 - off) * nB + (nA - off) - 1) // (nA - off)
                    while ib < min(tgt, nB) and prereq(b_list[ib]) <= i:
                        emit(b_list[ib])
                        ib += 1
            while ib < nB:
                emit(b_list[ib])
                ib += 1
    return _split_waits(nc) if split else nc


_NC_CACHE = {}


def _get_nc(st_tiles=5):
    if st_tiles not in _NC_CACHE:
        _NC_CACHE[st_tiles] = build_nc(st_tiles)
    return _NC_CACHE[st_tiles]


def make_in_maps(query, key, value, key_padding_mask, local_mask,
                 Wq, bq, Wk, bk, Wv, bv, Wo, bo, st_tiles=5):
    import ml_dtypes
    f = np.float32
    bf = ml_dtypes.bfloat16
    SQ = st_tiles * 128
    ident = np.eye(128, dtype=bf)
    in_maps = []
    for c in range(8):
        b, g = c // 2, c % 2
        gs = slice(g * G, (g + 1) * G)
        mask = np.asarray((key_padding_mask if g == 0 else local_mask)[b])
        sel = np.flatnonzero(~mask)
        ns = sel.size
        assert ns <= SQ, (ns, SQ)
        xk = np.zeros((E, SQ), dtype=bf)
        xk[:, :ns] = np.asarray(key[b]).T[:, sel]
        xv = np.zeros((E, SQ), dtype=bf)
        xv[:, :ns] = np.asarray(value[b]).T[:, sel]
        mbias = np.full(SQ, NEG, f)
        mbias[:ns] = 0.0
        cfm = np.concatenate([
            np.asarray(bq)[gs].astype(f).reshape(JT, 128).T,
            np.asarray(bk)[gs].astype(f).reshape(JT, 128).T,
            mbias.reshape(st_tiles, 128).T,
        ], axis=1)
        in_maps.append({
            "xqT": np.ascontiguousarray(np.asarray(query[b]).T, dtype=bf),
            "xkT": xk,
            "xvT": xv,
            "wqT": np.ascontiguousarray(np.asarray(Wq)[gs, :].T, dtype=bf),
            "wkT": np.ascontiguousarray(np.asarray(Wk)[gs, :].T, dtype=bf),
            "wvT": np.ascontiguousarray(np.asarray(Wv)[gs, :].T, dtype=bf),
            "woT": np.ascontiguousarray(np.asarray(Wo)[:, gs].T, dtype=bf),
            "cf": np.ascontiguousarray(cfm),
            "bvr": np.ascontiguousarray(np.asarray(bv)[gs].astype(bf).reshape(1, G)),
            "idn": ident,
        })
    return in_maps


def _needed_st_tiles(key_padding_mask, local_mask):
    worst = 0
    for c in range(8):
        b, g = c // 2, c % 2
        mask = np.asarray((key_padding_mask if g == 0 else local_mask)[b])
        worst = max(worst, int((~mask).sum()))
    return max(1, -(-worst // 128))


def kernel(query, key, value, key_padding_mask, local_mask,
           Wq, bq, Wk, bk, Wv, bv, Wo, bo, _trace=False, _tmpdir=None):
    from concourse.bass_utils import run_bass_kernel_spmd

    st = min(max(_needed_st_tiles(key_padding_mask, local_mask), 5), 8)
    nc = _get_nc(st)
    in_maps = make_in_maps(query, key, value, key_padding_mask, local_mask,
                           Wq, bq, Wk, bk, Wv, bv, Wo, bo, st_tiles=st)
    try:
        res = run_bass_kernel_spmd(nc, in_maps, list(range(8)),
                                   trace=_trace, tmpdir=_tmpdir)
    except Exception:
        # transient device/transport failures have been observed on the
        # axon path; one fresh attempt is cheap relative to a hard fail
        res = run_bass_kernel_spmd(nc, in_maps, list(range(8)),
                                   trace=_trace, tmpdir=_tmpdir)
    outs = [np.asarray(r["out"]) for r in res.results]
    full = np.stack([outs[2 * b] + outs[2 * b + 1] for b in range(B)])
    full += np.asarray(bo, dtype=np.float32)
    if _trace:
        kernel._last_exec_time_ns = res.exec_time_ns
        kernel._last_profile = res.profile_json
    return full.astype(np.float32)
